# revision 6
# baseline (speedup 1.0000x reference)
"""Trainium2 Bass kernel for ColorMatchingLoss (chamfer loss over YUV-sampled grids).

Math: x, y are [N=12288, B=64] grids sampled from YUV-converted images.
  d[i,j] = clip(|x_i - y_j|^2, 1e-5, 1e5)/64 + 1 - <x_i, y_j>/((|x_i|+eps)(|y_j|+eps))
  out = max(mean_i min_j d, mean_j min_i d)
For this input distribution |x_i - y_j|^2 in [2.9, 46.2], so the clip never binds.

Device strategy (8 cores, shard the 12288 x-rows, each core computes its
1536-row block of the distance matrix against the full y):
  Per 128x512 tile, PSUM accumulates d' = d - 1 via two fp16 matmuls:
    MM_e (K=66): [-x/32 ; xsq/64 ; 1]^T @ [y ; 1 ; ysq/64]  = |x-y|^2/64
    MM_t (K=64): [-(rx*x)]^T @ [ry*y]                       = -(cos-part)
  ScalarE copies PSUM->SBUF fp16; VectorE folds row-mins via a tt-min tree
  (2x f16 mode) and col-min partials via tt-min accumulate into a
  [128, 12288] running tile. Host finishes: col-min partition/core reduce,
  means, +1, max.
"""

import os

import numpy as np

N = 12288          # total grid rows (2 channels * 6144 samples)
B = 64             # feature dim (batch)
NCORES = 8
R = N // NCORES    # 1536 rows per core
MCHUNKS = R // 128  # 12
GROUP_COLS = 2048  # 4 psum banks per epilogue group
NGROUPS = N // GROUP_COLS  # 6

YUV_UV = np.array([[-0.14714119, -0.28886916, 0.43601035],
                   [0.61497538, -0.51496512, -0.10001026]], dtype=np.float32)
EPS = 1e-16

_compiled = None


def _build_bass(colmin_dma_ranges=(), act_split_cols=0, mm_n=512, loop_mult=1,
                s_bufs=3, sc_bufs=2, racc_bufs=2):
    """colmin_dma_ranges: set of group indices whose col-min accumulation
    runs on gpsimd DMA (CCE min) instead of VectorE.
    act_split_cols: leading columns of each group copied PSUM->SBUF by
    VectorE instead of ScalarE (load balance knob)."""
    from contextlib import ExitStack

    import concourse.bacc as bacc
    import concourse.bass as bass
    import concourse.tile as tile
    from concourse import mybir

    f16 = mybir.dt.float16
    f32 = mybir.dt.float32
    MIN = mybir.AluOpType.min

    nc = bacc.Bacc("TRN2", target_bir_lowering=False, debug=False,
                   num_devices=NCORES)

    lhsT_e_d = nc.dram_tensor("lhsT_e", [66, R], f16, kind="ExternalInput")
    lhsT_t_d = nc.dram_tensor("lhsT_t", [64, R], f16, kind="ExternalInput")
    rhs_e_d = nc.dram_tensor("rhs_e", [66, N], f16, kind="ExternalInput")
    rhs_t_d = nc.dram_tensor("rhs_t", [64, N], f16, kind="ExternalInput")
    rowmins_d = nc.dram_tensor("rowmins", [128, MCHUNKS], f32,
                               kind="ExternalOutput")
    colmins_d = nc.dram_tensor("colmins", [128, N], f16, kind="ExternalOutput")

    with tile.TileContext(nc) as tc, ExitStack() as ctx:
        consts = ctx.enter_context(tc.tile_pool(name="consts", bufs=1))
        spool = ctx.enter_context(tc.tile_pool(name="spool", bufs=s_bufs))
        scpool = ctx.enter_context(tc.tile_pool(name="scpool", bufs=sc_bufs))
        raccpool = ctx.enter_context(tc.tile_pool(name="racc", bufs=racc_bufs))
        pspool = ctx.enter_context(
            tc.tile_pool(name="psum", bufs=2, space=bass.MemorySpace.PSUM))

        lhsT_e = consts.tile([66, R], f16)
        lhsT_t = consts.tile([64, R], f16)
        rhs_e = consts.tile([66, N], f16)
        rhs_t = consts.tile([64, N], f16)
        colacc = consts.tile([128, N], f16)
        rowmins = consts.tile([128, MCHUNKS], f32)

        nc.sync.dma_start(lhsT_e[:], lhsT_e_d.ap())
        nc.sync.dma_start(lhsT_t[:], lhsT_t_d.ap())
        # chunk the big rhs DMAs so the first groups can start early
        for c in range(NGROUPS):
            cs = slice(c * GROUP_COLS, (c + 1) * GROUP_COLS)
            nc.sync.dma_start(rhs_e[:, cs], rhs_e_d.ap()[:, cs])
            nc.sync.dma_start(rhs_t[:, cs], rhs_t_d.ap()[:, cs])

        if colmin_dma_ranges:
            # DMA accum path needs colacc pre-set to +inf-ish
            for c in range(NGROUPS):
                if c in colmin_dma_ranges:
                    nc.gpsimd.memset(
                        colacc[:, c * GROUP_COLS:(c + 1) * GROUP_COLS], 60000.0)

        if loop_mult == 0:  # null kernel for overhead calibration
            nc.gpsimd.memset(rowmins[:], 0.0)
            nc.gpsimd.memset(colacc[:], 0.0)
            nc.sync.dma_start(colmins_d.ap(), colacc[:])

        for mi in range(MCHUNKS * loop_mult):
            m = mi % MCHUNKS
            ms = slice(m * 128, (m + 1) * 128)
            racc = raccpool.tile([128, 512], f16, tag="racc")
            for g in range(NGROUPS):
                n0 = g * GROUP_COLS
                ps = pspool.tile([128, GROUP_COLS], f32)
                for k in range(GROUP_COLS // mm_n):
                    ks = slice(k * mm_n, (k + 1) * mm_n)
                    ns = slice(n0 + k * mm_n, n0 + (k + 1) * mm_n)
                    nc.tensor.matmul(ps[:, ks], lhsT_e[:, ms], rhs_e[:, ns],
                                     start=True, stop=False)
                for k in range(GROUP_COLS // mm_n):
                    ks = slice(k * mm_n, (k + 1) * mm_n)
                    ns = slice(n0 + k * mm_n, n0 + (k + 1) * mm_n)
                    nc.tensor.matmul(ps[:, ks], lhsT_t[:, ms], rhs_t[:, ns],
                                     start=False, stop=True)

                s = spool.tile([128, GROUP_COLS], f16, tag="s")
                if act_split_cols:
                    nc.vector.tensor_copy(s[:, 0:act_split_cols],
                                          ps[:, 0:act_split_cols])
                    nc.scalar.activation(s[:, act_split_cols:],
                                         ps[:, act_split_cols:],
                                         mybir.ActivationFunctionType.Copy)
                else:
                    nc.scalar.activation(s[:], ps[:],
                                         mybir.ActivationFunctionType.Copy)

                # row-min: two tt-min tree levels into the running [128,512]
                sc1 = scpool.tile([128, 1024], f16, tag="sc1")
                nc.vector.tensor_tensor(sc1[:], s[:, 0:1024], s[:, 1024:2048],
                                        op=MIN)
                if g == 0:
                    nc.vector.tensor_tensor(racc[:], sc1[:, 0:512],
                                            sc1[:, 512:1024], op=MIN)
                else:
                    sc2 = scpool.tile([128, 512], f16, tag="sc2")
                    nc.vector.tensor_tensor(sc2[:], sc1[:, 0:512],
                                            sc1[:, 512:1024], op=MIN)
                    nc.vector.tensor_tensor(racc[:], racc[:], sc2[:], op=MIN)

                cs = slice(n0, n0 + GROUP_COLS)
                if g in colmin_dma_ranges:
                    nc.gpsimd.dma_start(colacc[:, cs], s[:],
                                        accum_op=MIN)
                elif m == 0:
                    nc.vector.tensor_copy(colacc[:, cs], s[:])
                else:
                    nc.vector.tensor_tensor(colacc[:, cs], colacc[:, cs],
                                            s[:], op=MIN)
                if mi == MCHUNKS * loop_mult - 1:
                    nc.sync.dma_start(colmins_d.ap()[:, cs], colacc[:, cs])

            nc.vector.tensor_reduce(rowmins[:, m:m + 1], racc[:],
                                    axis=mybir.AxisListType.X, op=MIN)

        nc.sync.dma_start(rowmins_d.ap(), rowmins[:])

    nc.compile()
    return nc


def _prepare_inputs(input_img, target_img, inds_y_input, inds_x_input,
                    inds_y_target, inds_x_target):
    input_img = np.asarray(input_img, dtype=np.float32)
    target_img = np.asarray(target_img, dtype=np.float32)
    iy_i = np.asarray(inds_y_input).astype(np.int64)
    ix_i = np.asarray(inds_x_input).astype(np.int64)
    iy_t = np.asarray(inds_y_target).astype(np.int64)
    ix_t = np.asarray(inds_x_target).astype(np.int64)

    def build_grid(img, iy, ix):
        g = (img[:, :, iy, ix] + 1.0) / 2.0          # [B,3,n]
        yuv = np.einsum('bcn,dc->bdn', g, YUV_UV)    # [B,2,n]
        return yuv.reshape(yuv.shape[0], -1).T.astype(np.float32)  # [2n,B]

    x = build_grid(input_img, iy_i, ix_i)   # [N, B]
    y = build_grid(target_img, iy_t, ix_t)  # [N, B]

    xsq = np.einsum('ij,ij->i', x, x)
    ysq = np.einsum('ij,ij->i', y, y)
    rx = 1.0 / (np.sqrt(xsq) + EPS)
    ry = 1.0 / (np.sqrt(ysq) + EPS)

    f16 = np.float16
    rhs_e = np.empty((66, N), dtype=f16)
    rhs_e[0:64] = y.T.astype(f16)
    rhs_e[64] = 1.0
    rhs_e[65] = (ysq / 64.0).astype(f16)
    rhs_t = (y * ry[:, None]).T.astype(f16)

    lhsT_e_full = np.empty((66, N), dtype=f16)
    lhsT_e_full[0:64] = (-x / 32.0).T.astype(f16)
    lhsT_e_full[64] = (xsq / 64.0).astype(f16)
    lhsT_e_full[65] = 1.0
    lhsT_t_full = (-(x * rx[:, None])).T.astype(f16)

    in_maps = []
    for c in range(NCORES):
        rs = slice(c * R, (c + 1) * R)
        in_maps.append({
            "lhsT_e": np.ascontiguousarray(lhsT_e_full[:, rs]),
            "lhsT_t": np.ascontiguousarray(lhsT_t_full[:, rs]),
            "rhs_e": rhs_e,
            "rhs_t": rhs_t,
        })
    return in_maps


def _combine(results):
    rowmin_all = np.concatenate(
        [r["rowmins"].T.reshape(-1) for r in results])        # [N]
    colmin_stack = np.stack([r["colmins"] for r in results])  # [8,128,N]
    colmin = colmin_stack.astype(np.float32).min(axis=(0, 1))  # [N]
    m1 = 1.0 + rowmin_all.mean()
    m2 = 1.0 + colmin.mean()
    return np.asarray(np.float32(max(m1, m2)))


def kernel(input_img, target_img, inds_y_input, inds_x_input,
           inds_y_target, inds_x_target):
    global _compiled
    import time

    from concourse import bass_utils

    if _compiled is None:
        _compiled = _build_bass()
    nc = _compiled

    in_maps = _prepare_inputs(input_img, target_img, inds_y_input,
                              inds_x_input, inds_y_target, inds_x_target)
    # Retry: a previously-crashed tenant can leave the NeuronCore wedged
    # (NRT_EXEC_UNIT_UNRECOVERABLE) for one execution attempt before it
    # self-clears; a fresh attempt then succeeds.
    last_err = None
    for attempt in range(4):
        try:
            res = bass_utils.run_bass_kernel_spmd(
                nc, in_maps, core_ids=list(range(NCORES)))
            return _combine(res.results)
        except Exception as e:  # noqa: BLE001
            last_err = e
            time.sleep(3.0)
    raise last_err
